# revision 37
# baseline (speedup 1.0000x reference)
"""Self-contained Trainium2 Bass kernel for nn_Attention_87282325389426.

GQA attention with "raw reshape" (scrambled) head semantics:
  B=2, S=2048, D=2048, HQ=16, HK=HV=4, DQK=128, DV=512.

Sharding: the raw-reshape semantics decompose exactly into B*HK = 8
independent (batch, kv-group) units -> one per NeuronCore, zero collectives.
Core i handles (b, k) = (i//4, i%4):
  - queries  : x rows in spans {(4g+k)*128 : g=0..3}   (512 rows)
  - keys/vals: x rows [512k, 512(k+1))                 (512 rows)
  - output   : full 2048-wide rows for the 4 query spans (disjoint across cores)

Per-core pipeline (layouts chosen so NO on-chip transposes are needed):
  QT[d, (g,ds,h)]  = WQ_hblock^T @ xT_q      (projection emits transposed Q)
  KT[d, (h,ds)]    = WK_hblock^T @ xT_kv
  V  [s, (h,dv)]   natural
  ST[t'=(h,ds'), t=(ds,h')] = KT_tile^T @ QT_block  (scores transposed, bf16)
  P = exp(ST/sqrt(128)) * causal_mask  (mask only on diagonal tiles; upper
      blocks skipped entirely -> ~47% of score/PV work elided)
  OT[dv, t] += V_chunk^T @ P_chunk   (PV emits transposed O directly; row
      sums replicated across partitions via ones[128,128] matmul on the PE)
  y[s, n] = (OT/rowsum via strided APs) @ W0 + b0    (W0 in bf16)
"""

import math

import numpy as np

import concourse.bass as bass  # noqa: F401
import concourse.mybir as mybir
import concourse.tile as tile
from concourse import bacc
from concourse.bass_utils import run_bass_kernel_spmd

F32 = mybir.dt.float32
BF16 = mybir.dt.bfloat16
EXP = mybir.ActivationFunctionType.Exp
GE = mybir.AluOpType.is_ge

D = 2048
SCALE = 1.0 / math.sqrt(128.0)


def _body(tc, ctx, xtq, xtkv, wq, wk, wv, w0, bq, bk, bv, b0, out):
    nc = tc.nc

    pers = ctx.enter_context(tc.tile_pool(name="pers", bufs=1))
    qt = pers.tile([128, 4 * 2048], BF16)     # free = g*2048 + ds*16 + h
    kt = pers.tile([128, 4 * 512], BF16)      # free = h*512 + ds
    vsb = pers.tile([128, 4, 2048], BF16)     # [s-part, s-tile, (h,dv)]
    ones = pers.tile([128, 128], BF16)
    maskt = pers.tile([128, 4, 512], BF16)    # per-h diagonal causal mask
    mask1 = pers.tile([128, 512], BF16)
    bq_sb = pers.tile([128, 16], F32)
    bk_sb = pers.tile([128, 4], F32)
    bv_sb = pers.tile([128, D], F32)
    b0_sb = pers.tile([128, D], F32)

    nc.vector.memset(ones, 1.0)
    nc.vector.memset(mask1, 1.0)
    nc.gpsimd.dma_start(out=bq_sb, in_=bq[:, :])
    nc.gpsimd.dma_start(out=bk_sb, in_=bk[:, :])
    # mask[p, h, n] = 1.0 if n >= 4p + h else 0  (valid key iff t' <= t)
    for h in range(4):
        nc.gpsimd.affine_select(
            out=maskt[:, h, :], in_=mask1,
            pattern=[[1, 512]], compare_op=GE, fill=0.0,
            base=-h, channel_multiplier=-4,
        )

    # ---------------- projections (all bf16 operands) ----------------
    qt_w = qt.rearrange("p (g s h) -> p g s h", g=4, h=16)
    with (
        tc.tile_pool(name="xp", bufs=1) as xp,
        tc.tile_pool(name="ws", bufs=10) as ws,
        tc.tile_pool(name="pps", bufs=8, space="PSUM") as pps,
    ):
        xtq_sb = xp.tile([128, 16, 512], BF16)
        xtkv_sb = xp.tile([128, 16, 512], BF16)
        xtq_r = xtq[:, :].rearrange("(cc p) n -> p cc n", p=128)
        xtkv_r = xtkv[:, :].rearrange("(cc p) n -> p cc n", p=128)
        nc.sync.dma_start(out=xtq_sb[:, 0:2, :], in_=xtq_r[:, 0:2, :])

        # Q: two half-column passes over WQ, 8 psum banks each
        wq_c = wq[:, :].rearrange("(cc p) n -> p cc n", p=128)
        for hp in range(2):
            pq = [pps.tile([128, 512], F32, tag="pj", name=f"pq{hp}_{i}")
                  for i in range(8)]
            for cc in range(16):
                wt = ws.tile([128, 1024], BF16, tag="wq")
                nc.sync.dma_start(out=wt, in_=wq_c[:, cc, hp * 1024:(hp + 1) * 1024])
                if hp == 0 and cc == 0:
                    nc.sync.dma_start(out=xtq_sb[:, 2:6, :], in_=xtq_r[:, 2:6, :])
                if hp == 0 and cc == 1:
                    nc.sync.dma_start(out=xtq_sb[:, 6:16, :], in_=xtq_r[:, 6:16, :])
                if hp == 1 and cc == 0:
                    nc.sync.dma_start(out=xtkv_sb[:, 0:8, :], in_=xtkv_r[:, 0:8, :])
                if hp == 1 and cc == 8:
                    nc.sync.dma_start(out=xtkv_sb[:, 8:16, :], in_=xtkv_r[:, 8:16, :])
                for hi in range(8):
                    nc.tensor.matmul(pq[hi], wt[:, hi * 128:(hi + 1) * 128],
                                     xtq_sb[:, cc, :],
                                     start=(cc == 0), stop=(cc == 15))
            for hi in range(8):
                h = hp * 8 + hi
                nc.vector.tensor_scalar_add(
                    qt_w[:, :, :, h],
                    pq[hi].rearrange("p (g s) -> p g s", g=4),
                    bq_sb[:, h:h + 1])

        # K: one pass, 4 banks
        wk_c = wk[:, :].rearrange("(cc p) n -> p cc n", p=128)
        pk = [pps.tile([128, 512], F32, tag="pj", name=f"pk_{i}") for i in range(4)]
        for cc in range(16):
            wt = ws.tile([128, 512], BF16, tag="wk")
            nc.sync.dma_start(out=wt, in_=wk_c[:, cc, :])
            for h in range(4):
                nc.tensor.matmul(pk[h], wt[:, h * 128:(h + 1) * 128],
                                 xtkv_sb[:, cc, :],
                                 start=(cc == 0), stop=(cc == 15))
        for h in range(4):
            nc.vector.tensor_scalar_add(kt[:, h * 512:(h + 1) * 512], pk[h],
                                        bk_sb[:, h:h + 1])

        nc.sync.dma_start(out=bv_sb, in_=bv[0:1, :].partition_broadcast(128))
        nc.sync.dma_start(out=b0_sb, in_=b0[0:1, :].partition_broadcast(128))

        # V natural [s, (h,dv)]; four (st-pair, nb-pair) passes, 4 banks each
        wv_c = wv[:, :].rearrange("(cc p) n -> p cc n", p=128)
        for stp in range(2):
            for nbp in range(2):
                psv = [pps.tile([128, 512], F32, tag="pj", name=f"psv{stp}{nbp}_{i}")
                       for i in range(4)]
                for cc in range(16):
                    wt = ws.tile([128, 1024], BF16, tag="wv")
                    nc.sync.dma_start(
                        out=wt, in_=wv_c[:, cc, nbp * 1024:(nbp + 1) * 1024])
                    for sti in range(2):
                        st = stp * 2 + sti
                        for nbi in range(2):
                            nc.tensor.matmul(
                                psv[sti * 2 + nbi],
                                xtkv_sb[:, cc, st * 128:(st + 1) * 128],
                                wt[:, nbi * 512:(nbi + 1) * 512],
                                start=(cc == 0), stop=(cc == 15))
                for sti in range(2):
                    st = stp * 2 + sti
                    for nbi in range(2):
                        nb = nbp * 2 + nbi
                        nc.vector.tensor_add(
                            vsb[:, st, nb * 512:(nb + 1) * 512],
                            psv[sti * 2 + nbi],
                            bv_sb[:, nb * 512:(nb + 1) * 512])

    # ---------------- attention ----------------
    otpool = ctx.enter_context(tc.tile_pool(name="otpool", bufs=1))
    ot = otpool.tile([128, 4, 4, 2048], BF16)   # [dv-part, g, dvc, t]
    with (
        tc.tile_pool(name="pp", bufs=28) as ppool,
        tc.tile_pool(name="rp", bufs=3) as rp,
        tc.tile_pool(name="stps", bufs=3, space="PSUM") as stps,
        tc.tile_pool(name="otps", bufs=2, space="PSUM") as otps,
        tc.tile_pool(name="sumps", bufs=1, space="PSUM") as sumps,
    ):
        for g in range(4):
            for qb in range(4):
                # contiguous query block, cols n = ds*16 + h
                rhs_q = qt[:, g * 2048 + qb * 512: g * 2048 + (qb + 1) * 512]
                ptiles = []
                for m2 in range(qb + 1):
                    for h in range(4):
                        ps = stps.tile([128, 512], F32)
                        nc.tensor.matmul(
                            ps,
                            kt[:, h * 512 + m2 * 128: h * 512 + (m2 + 1) * 128],
                            rhs_q, start=True, stop=True)
                        pt = ppool.tile([128, 512], BF16)
                        nc.scalar.activation(pt, ps, EXP, scale=SCALE)
                        if m2 == qb:
                            nc.vector.tensor_mul(pt, pt, maskt[:, h, :])
                        ptiles.append((m2, h, pt))
                n = len(ptiles)
                otpA = otps.tile([128, 2, 512], F32, tag="otp", name=f"otpA_{g}_{qb}")
                smp = sumps.tile([128, 512], F32)
                for i, (m2, h, pt) in enumerate(ptiles):
                    first, last = (i == 0), (i == n - 1)
                    for dvc in range(2):
                        nc.tensor.matmul(
                            otpA[:, dvc, :],
                            vsb[:, m2, h * 512 + dvc * 128: h * 512 + (dvc + 1) * 128],
                            pt, start=first, stop=last)
                    nc.tensor.matmul(smp, ones, pt, start=first, stop=last)
                # reciprocal overlaps PV pass B on the PE
                rcb = rp.tile([128, 512], F32, tag="rcb")
                nc.vector.reciprocal(rcb, smp)
                otpB = otps.tile([128, 2, 512], F32, tag="otp", name=f"otpB_{g}_{qb}")
                for i, (m2, h, pt) in enumerate(ptiles):
                    first, last = (i == 0), (i == n - 1)
                    for dvc in range(2, 4):
                        nc.tensor.matmul(
                            otpB[:, dvc - 2, :],
                            vsb[:, m2, h * 512 + dvc * 128: h * 512 + (dvc + 1) * 128],
                            pt, start=first, stop=last)
                for dvc in range(2):
                    nc.vector.tensor_mul(
                        ot[:, g, dvc, qb * 512:(qb + 1) * 512],
                        otpA[:, dvc, :], rcb)
                for dvc in range(2, 4):
                    nc.vector.tensor_mul(
                        ot[:, g, dvc, qb * 512:(qb + 1) * 512],
                        otpB[:, dvc - 2, :], rcb)

    ones_row = pers.tile([1, 128], BF16)
    nc.vector.memset(ones_row, 1.0)
    b0bf = pers.tile([1, D], BF16)
    nc.vector.tensor_copy(b0bf, b0_sb[0:1, :])

    # ---------------- output GEMM ----------------
    w0_r = w0[:, :].rearrange("(cj p) n -> p cj n", p=128)   # cj = j*4 + dvc
    with (
        tc.tile_pool(name="w0s", bufs=10) as w0s,
        tc.tile_pool(name="yp", bufs=4) as yp,
        tc.tile_pool(name="yps", bufs=8, space="PSUM") as yps,
    ):
        for nbp in range(2):
            ypsum = [yps.tile([128, 512], F32, tag="y", name=f"ypsum{nbp}_{i}")
                     for i in range(8)]
            # bias row: y += ones_col^T(K=1) @ b0_row
            for g in range(4):
                for nbi in range(2):
                    nb = nbp * 2 + nbi
                    nc.tensor.matmul(
                        ypsum[g * 2 + nbi], ones_row,
                        b0bf[0:1, nb * 512:(nb + 1) * 512],
                        start=True, stop=False)
            for cj in range(64):
                j, dvc = cj // 4, cj % 4
                wt = w0s.tile([128, 1024], BF16)
                nc.sync.dma_start(out=wt, in_=w0_r[:, cj, nbp * 1024:(nbp + 1) * 1024])
                for g in range(4):
                    lt = ot[:, g, dvc, :].rearrange("p (s j) -> p s j", j=16)[:, :, j]
                    for nbi in range(2):
                        nc.tensor.matmul(
                            ypsum[g * 2 + nbi], lt, wt[:, nbi * 512:(nbi + 1) * 512],
                            start=False, stop=(cj == 63))
            for g in range(4):
                for nbi in range(2):
                    nb = nbp * 2 + nbi
                    yt = yp.tile([128, 512], F32)
                    nc.vector.tensor_copy(yt, ypsum[g * 2 + nbi])
                    nc.sync.dma_start(
                        out=out[g * 128:(g + 1) * 128, nb * 512:(nb + 1) * 512],
                        in_=yt)


def build_graph():
    nc = bacc.Bacc(None, target_bir_lowering=False)
    xtq = nc.declare_dram_parameter("xtq", [D, 512], BF16, isOutput=False)
    xtkv = nc.declare_dram_parameter("xtkv", [D, 512], BF16, isOutput=False)
    wq = nc.declare_dram_parameter("wq", [D, D], BF16, isOutput=False)
    wk = nc.declare_dram_parameter("wk", [D, 512], BF16, isOutput=False)
    wv = nc.declare_dram_parameter("wv", [D, D], BF16, isOutput=False)
    w0 = nc.declare_dram_parameter("w0", [8192, D], BF16, isOutput=False)
    bq = nc.declare_dram_parameter("bq", [128, 16], F32, isOutput=False)
    bk = nc.declare_dram_parameter("bk", [128, 4], F32, isOutput=False)
    bv = nc.declare_dram_parameter("bv", [1, D], F32, isOutput=False)
    b0 = nc.declare_dram_parameter("b0", [1, D], F32, isOutput=False)
    out = nc.declare_dram_parameter("out", [512, D], F32, isOutput=True)
    from contextlib import ExitStack
    with tile.TileContext(nc) as tc, ExitStack() as ctx:
        _body(tc, ctx, xtq, xtkv, wq, wk, wv, w0, bq, bk, bv, b0, out)
    nc.finalize()
    return nc


_CACHE = {}


def _get_nc():
    if "nc" not in _CACHE:
        _CACHE["nc"] = build_graph()
    return _CACHE["nc"]


def _prep_in_maps(x, WQ, bQ, WK, bK, WV, bV, W0, b0):
    bf16 = mybir.dt.np(BF16)
    x = np.asarray(x, np.float32)
    w0_bf = np.ascontiguousarray(np.asarray(W0, np.float32).astype(bf16))
    wq_bf = np.ascontiguousarray(np.asarray(WQ, np.float32).astype(bf16))
    wk_bf = np.ascontiguousarray(np.asarray(WK, np.float32).astype(bf16))
    wv_bf = np.ascontiguousarray(np.asarray(WV, np.float32).astype(bf16))
    bq_r = np.ascontiguousarray(np.asarray(bQ, np.float32).reshape(16, 128).T)
    bk_r = np.ascontiguousarray(np.asarray(bK, np.float32).reshape(4, 128).T)
    bv_r = np.ascontiguousarray(np.asarray(bV, np.float32).reshape(1, D))
    b0_r = np.ascontiguousarray(np.asarray(b0, np.float32).reshape(1, D))
    in_maps = []
    for core in range(8):
        b, k = core // 4, core % 4
        q_rows = np.concatenate(
            [np.arange((4 * g + k) * 128, (4 * g + k + 1) * 128) for g in range(4)])
        xtq = np.ascontiguousarray(x[b, q_rows, :].T.astype(bf16))
        xtkv = np.ascontiguousarray(x[b, 512 * k:512 * (k + 1), :].T.astype(bf16))
        in_maps.append({
            "xtq": xtq, "xtkv": xtkv, "wq": wq_bf, "wk": wk_bf, "wv": wv_bf,
            "w0": w0_bf, "bq": bq_r, "bk": bk_r, "bv": bv_r, "b0": b0_r,
        })
    return in_maps


def _install_ntff_hook_shim():
    """The image's antenv lacks axon_hooks; provide it so trace=True works."""
    import sys
    import types
    if "antenv.axon_hooks" in sys.modules:
        return
    mod = types.ModuleType("antenv.axon_hooks")
    mod._hook = None

    def set_axon_ntff_profile_hook(h):
        mod._hook = h

    def get_axon_ntff_profile_hook():
        return mod._hook

    mod.set_axon_ntff_profile_hook = set_axon_ntff_profile_hook
    mod.get_axon_ntff_profile_hook = get_axon_ntff_profile_hook
    sys.modules["antenv.axon_hooks"] = mod
    try:
        from trn_agent_boot.trn_boot import _ntff_profile_via_ctypes
        mod._hook = _ntff_profile_via_ctypes("/opt/axon/libaxon_pjrt.so")
    except Exception as e:  # pragma: no cover
        print("ntff shim: hook unavailable:", e)


def run(inputs, trace=False, tmpdir=None, return_res=False):
    """Run on 8 cores; returns (full_output, exec_time_ns_or_None)."""
    if trace:
        _install_ntff_hook_shim()
    in_maps = _prep_in_maps(
        inputs["x"], inputs["WQ"], inputs["bQ"], inputs["WK"], inputs["bK"],
        inputs["WV"], inputs["bV"], inputs["W0"], inputs["b0"])
    res = run_bass_kernel_spmd(_get_nc(), in_maps, core_ids=list(range(8)), trace=trace,
                               tmpdir=tmpdir)
    full = np.zeros((2, 2048, 2048), np.float32)
    for core in range(8):
        b, k = core // 4, core % 4
        co = res.results[core]["out"]
        for g in range(4):
            full[b, (4 * g + k) * 128:(4 * g + k + 1) * 128, :] = co[g * 128:(g + 1) * 128, :]
    if return_res:
        return full, res
    return full, res.exec_time_ns


def kernel(**inputs):
    out, _ = run(inputs, trace=False)
    return out
